# revision 50
# baseline (speedup 1.0000x reference)
# GQA causal attention with RoPE on 8 TRN2 NeuronCores (tensor-parallel over heads).
#
# Reference computation (B=2, S=4096, D=2048, H=16 heads, KVH=4 kv heads, HD=128):
#   q/k/v projections -> RoPE on q,k -> causal GQA attention -> o_proj.
#
# Sharding (per hint): core c owns Q heads {2c, 2c+1}; kv head c//2 is split
# across the core pair -- the even core projects K (with RoPE), the odd core
# projects V (same SPMD program: its RoPE tables are cos=1/sin=0), and a
# pairwise AllGather swaps the halves (issued per sequence-half, so the first
# exchange overlaps the rest of that batch's projections). Each core runs
# causal attention for its 2 heads over the full sequence, producing the
# transposed context [HD, B*S] per head. A per-head AllToAll (chunk j ->
# core j) redistributes so each core holds all 16 heads for its 1/8 slice of
# the B*S rows; the head-0 exchange overlaps the head-1 attention, and o_proj
# runs two-pass (all even-head partial sums accumulate with PSUM groups held
# open before any odd-head matmul) so its first ~half is independent of the
# final exchange's latency.
#
# Phase B (attention) is software-pipelined one kv-pair ahead: the NEXT
# pair's score matmuls are emitted BEFORE this pair's attn@V, so the PE's
# in-order stream fills the exp-latency window and the scalar engine (the
# phase-B bottleneck at ~1.04us per [128,1024] exp) runs back-to-back.
# Causal masking costs no DVE work: each diagonal 128-block gets a -1e9
# strict-upper-triangle added INSIDE the score-PSUM accumulation group by a
# single 128-column matmul (lhsT = strict-lower -1e9 triangle, rhs =
# identity), so exp underflows to exact 0 there; fully-masked column ranges
# are simply never exp'd -- the diagonal-pair et tiles are dedicated
# (bufs=1 per type) with those ranges memzero'd ONCE before the loop.
# The softmax denominator accumulates per kv-block on the DVE in bf16 and
# collapses across partitions with a ones-vector matmul per q-block; the
# reciprocal/broadcast run in bf16 (213ns broadcast matmul) sharing one
# PSUM bank (the DVE cannot read two PSUM operands, so the broadcast stages
# through SBUF before the normalize multiply). Matmul operands are bf16
# (1 cyc/row); PSUM accumulation f32.
# End-to-end rel err vs the f32 reference: ~5.3e-3 (measured on hardware).

import math
import sys

for _p in ("/opt/trn_rl_repo",):
    if _p not in sys.path:
        sys.path.insert(0, _p)

import numpy as np
import ml_dtypes

B = 2
S = 4096
D = 2048
H = 16
KVH = 4
HD = 128
N_CORES = 8
BS = B * S                  # 8192 flattened rows
SHARD = BS // N_CORES       # 1024 output rows per core
HPC = H // N_CORES          # 2 q heads per core
SCALE = 1.0 / math.sqrt(HD)

SQ = 512                    # q-block (matmul free dim)
KV = 128                    # kv-block (psum partition dim)
DCH = D // 128              # 16 contraction chunks for the projections
NB = S // SQ                # 8 q-blocks per batch
NKV_B = S // KV             # 32 kv-blocks per batch
DIAG = SQ // KV             # 4 kv-blocks per q-block on the causal diagonal

BF16 = ml_dtypes.bfloat16

_CACHE = {}
PHASE_MARKS = []


def _mark(nc, phase):
    try:
        PHASE_MARKS.append((phase, int(nc._state.next_id())))
    except Exception:
        pass


def _build(sim_mode=False):
    import concourse.mybir as mybir
    import concourse.tile as tile
    from concourse import bacc

    dt = mybir.dt
    nc = bacc.Bacc("TRN2", target_bir_lowering=False, debug=False,
                   enable_asserts=True, num_devices=N_CORES)

    # ---- external inputs (per-core shards supplied via in_maps) ----
    xT = nc.dram_tensor("xT", [D, BS], dt.bfloat16, kind="ExternalInput")
    cosT = nc.dram_tensor("cosT", [HD, S], dt.bfloat16, kind="ExternalInput")
    sinTs = nc.dram_tensor("sinTs", [HD, S], dt.bfloat16, kind="ExternalInput")
    wq = nc.dram_tensor("wq", [D, HPC * HD], dt.bfloat16, kind="ExternalInput")
    # kv weight: even cores carry their kv-head's K columns, odd cores the V
    # columns; the pair exchanges results (same SPMD program, different data)
    wkv = nc.dram_tensor("wkv", [D, HD], dt.bfloat16, kind="ExternalInput")
    coskv = nc.dram_tensor("coskv", [HD, S], dt.bfloat16, kind="ExternalInput")
    sinkv = nc.dram_tensor("sinkv", [HD, S], dt.bfloat16, kind="ExternalInput")
    wo = nc.dram_tensor("wo", [D, D], dt.bfloat16, kind="ExternalInput")
    tri = nc.dram_tensor("tri", [128, 128], dt.bfloat16, kind="ExternalInput")
    ident = nc.dram_tensor("ident", [128, 128], dt.bfloat16, kind="ExternalInput")
    onesb = nc.dram_tensor("onesb", [128, 1], dt.bfloat16, kind="ExternalInput")
    onesf = nc.dram_tensor("onesf", [1, 128], dt.bfloat16, kind="ExternalInput")

    out = nc.dram_tensor("out", [SHARD, D], dt.float32, kind="ExternalOutput")

    # ---- internal DRAM for the pairwise k/v exchange ----
    # half-major so each sequence-half exchanges as soon as phase A produces it
    ktv_dram = [nc.dram_tensor(f"ktv{b}", [2, HD, S // 2], dt.bfloat16)
                for b in range(B)]
    kv_pair = [nc.dram_tensor(f"kvp{b}", [2, 2, HD, S // 2], dt.bfloat16)
               for b in range(B)]

    # ---- internal DRAM for the AllToAll (one buffer per local head, so the
    # h=0 exchange overlaps the h=1 attention) ----
    ao_in = [nc.dram_tensor(f"ao_in{h}", [N_CORES, HD, SHARD], dt.bfloat16)
             for h in range(HPC)]
    ao_ex = [nc.dram_tensor(f"ao_ex{h}", [N_CORES, HD, SHARD], dt.bfloat16)
             for h in range(HPC)]
    if sim_mode:
        ao_ex = ao_in   # single-core TimelineSim: same DMA pattern

    with tile.TileContext(nc) as tc:
        with tc.tile_pool(name="persist", bufs=1) as pp:
            tri_sb = pp.tile([128, 128], dt.bfloat16, name="tri_sb")
            id_sb = pp.tile([128, 128], dt.bfloat16, name="id_sb")
            ob_sb = pp.tile([128, 1], dt.bfloat16, name="ob_sb")
            of_sb = pp.tile([1, 128], dt.bfloat16, name="of_sb")

            # exchanged o_proj context: streamed into SBUF DURING phase B.
            # Manual enter/exit: pdp must outlive qkvp (strict pool stack
            # order) but phase D sits lexically after qkvp's block.
            _pdp_cm = tc.tile_pool(name="pdp", bufs=1)
            pdp = _pdp_cm.__enter__()
            lt_all = [pdp.tile([128, SHARD // 128, N_CORES, 128],
                               dt.bfloat16, name="lt_all0")]
            _qkv_cm = tc.tile_pool(name="qkv", bufs=2)
            qkvp = _qkv_cm.__enter__()
            if True:
              qts, kts, vns, vtbs = {}, {}, {}, {}
              xs_pre = {}
              with tc.tile_pool(name="ropep", bufs=1) as rp, \
                   tc.tile_pool(name="pa", bufs=2) as pa, \
                   tc.tile_pool(name="pax", bufs=3) as pax, \
                   tc.tile_pool(name="paps", bufs=2, space="PSUM") as paps:
                cos_sb = rp.tile([HD, S], dt.bfloat16, name="cos_sb")
                sin_sb = rp.tile([HD, S], dt.bfloat16, name="sin_sb")
                coskv_sb = rp.tile([HD, S], dt.bfloat16, name="coskv_sb")
                sinkv_sb = rp.tile([HD, S], dt.bfloat16, name="sinkv_sb")
                wq_sb = rp.tile([128, DCH, HPC * HD], dt.bfloat16,
                                name="wq_sb")
                wkv_sb = rp.tile([128, DCH, HD], dt.bfloat16, name="wkv_sb")

                # first contraction chunks ahead of the rest so the opening
                # matmuls aren't stuck behind the full weight DMA
                wqr = wq[:].rearrange("(k p) m -> p k m", p=128)
                wkvr = wkv[:].rearrange("(k p) m -> p k m", p=128)
                nc.sync.dma_start(out=wq_sb[:, 0:2, :], in_=wqr[:, 0:2, :])
                nc.sync.dma_start(out=wkv_sb[:, 0:2, :], in_=wkvr[:, 0:2, :])
                nc.sync.dma_start(out=wq_sb[:, 2:DCH, :], in_=wqr[:, 2:DCH, :])
                nc.sync.dma_start(out=wkv_sb[:, 2:DCH, :],
                                  in_=wkvr[:, 2:DCH, :])

                def load_xs(bb, sj, split=1):
                    t = pax.tile([128, DCH, SQ], dt.bfloat16, name="xs", tag="xs")
                    xr = xT[:, bb * S + sj * SQ:bb * S + sj * SQ + SQ].rearrange(
                        "(k p) n -> p k n", p=128)
                    step = DCH // split
                    for h0 in range(0, DCH, step):
                        nc.sync.dma_start(out=t[:, h0:h0 + step, :],
                                          in_=xr[:, h0:h0 + step, :])
                    return t

                for b in range(B):
                    qt = qkvp.tile([HD, HPC, S], dt.bfloat16, name=f"qt{b}", tag="qt")
                    kt = qkvp.tile([HD, S], dt.bfloat16, name=f"kt{b}", tag="kt")
                    vn = qkvp.tile([128, NKV_B, HD], dt.bfloat16, name=f"vn{b}",
                                   tag="vn")
                    qts[b], kts[b], vns[b] = qt, kt, vn
                    _mark(nc, f"A{b}")
                    # ------- phase A: q/k/v projections + RoPE (batch b) -------
                    # k-or-v staging per sequence-half (each half ships to the
                    # pair exchange as soon as it completes)
                    ktv_sb = None
                    for si in range(NB):
                        if si % (NB // 2) == 0:
                            ktv_sb = pa.tile([HD, S // 2], dt.bfloat16,
                                             name=f"ktv{b}", tag="ktv")
                        s0 = b * S + si * SQ          # column into xT
                        l0 = si * SQ                  # column into cos/sin
                        pq0 = paps.tile([128, SQ], dt.float32, name="pq0",
                                        tag="pq0")
                        pq1 = paps.tile([128, SQ], dt.float32, name="pq1",
                                        tag="pq1")
                        pk = paps.tile([128, SQ], dt.float32, name="pk", tag="pk")
                        xs = xs_pre.pop((b, si), None)
                        if xs is None:
                            xs = load_xs(b, si, split=(4 if (b == 0 and si == 0)
                                                       else 1))
                        if b == 0 and si == 0:
                            # rope tables: first use is ~13us in; emit after
                            # xs0 so the first matmuls aren't starved
                            nc.sync.dma_start(out=cos_sb[:], in_=cosT[:])
                            nc.sync.dma_start(out=sin_sb[:], in_=sinTs[:])
                            nc.sync.dma_start(out=coskv_sb[:], in_=coskv[:])
                            nc.sync.dma_start(out=sinkv_sb[:], in_=sinkv[:])
                            nc.sync.dma_start(out=tri_sb[:], in_=tri[:])
                            nc.sync.dma_start(out=id_sb[:], in_=ident[:])
                            nc.sync.dma_start(out=ob_sb[:], in_=onesb[:])
                            nc.sync.dma_start(out=of_sb[:], in_=onesf[:])
                        for k in range(DCH):
                            st = (k == 0)
                            sp = (k == DCH - 1)
                            nc.tensor.matmul(pq0[:], lhsT=wq_sb[:, k, 0:128],
                                             rhs=xs[:, k, :], start=st, stop=sp)
                            nc.tensor.matmul(pq1[:], lhsT=wq_sb[:, k, 128:256],
                                             rhs=xs[:, k, :], start=st, stop=sp)
                            nc.tensor.matmul(pk[:], lhsT=wkv_sb[:, k, :],
                                             rhs=xs[:, k, :], start=st, stop=sp)
                        # RoPE: dest = p*cos + rot(p)*sin_signed.  The kv
                        # projection uses per-core tables: real rope on
                        # even (K) cores, identity (cos=1,sin=0) on odd
                        # (V) cores -- same program, different data.
                        lh = l0 % (S // 2)        # column within the half
                        for ph, cs, sn, dest in (
                                (pq0, cos_sb, sin_sb, qt[:, 0, l0:l0 + SQ]),
                                (pq1, cos_sb, sin_sb, qt[:, 1, l0:l0 + SQ]),
                                (pk, coskv_sb, sinkv_sb,
                                 ktv_sb[:, lh:lh + SQ])):
                            t1 = pa.tile([128, SQ], dt.float32, name="t1",
                                         tag="t1")
                            t2 = pa.tile([128, SQ], dt.float32, name="t2",
                                         tag="t2")
                            nc.vector.tensor_mul(out=t1[:], in0=ph[:],
                                                 in1=cs[:, l0:l0 + SQ])
                            nc.vector.tensor_mul(out=t2[0:64, :],
                                                 in0=ph[64:128, :],
                                                 in1=sn[0:64, l0:l0 + SQ])
                            nc.vector.tensor_mul(out=t2[64:128, :],
                                                 in0=ph[0:64, :],
                                                 in1=sn[64:128, l0:l0 + SQ])
                            nc.vector.tensor_add(out=dest, in0=t1[:], in1=t2[:])
                        # ship each completed sequence-half of this core's
                        # k-or-v and exchange it with the partner (half 0
                        # overlaps the rest of this batch's A)
                        if si in (NB // 2 - 1, NB - 1):
                            hx = si // (NB // 2)
                            nc.sync.dma_start(out=ktv_dram[b][hx],
                                              in_=ktv_sb[:])
                            if not sim_mode:
                                nc.gpsimd.collective_compute(
                                    "AllGather", mybir.AluOpType.bypass,
                                    replica_groups=[[2 * g, 2 * g + 1]
                                                    for g in
                                                    range(N_CORES // 2)],
                                    ins=[ktv_dram[b][hx]],
                                    outs=[kv_pair[b][hx]])
                            else:
                                nc.sync.dma_start(out=kv_pair[b][hx, 0],
                                                  in_=ktv_dram[b][hx])
                                nc.sync.dma_start(out=kv_pair[b][hx, 1],
                                                  in_=ktv_dram[b][hx])
                    # prefetch the next batch's first activation block
                    if b + 1 < B:
                        xs_pre[(b + 1, 0)] = load_xs(b + 1, 0)

                # prefetch ALL exchanged K plus batch 0's V (after the loop:
                # phase A's xT loads own the DMA engines until here; batch
                # 1's V streams into the same buffer after b0's transposes)
                vtb0 = qkvp.tile([HD, S], dt.bfloat16, name="vtb", tag="vtb",
                                 bufs=1)
                vtbs[0] = vtbs[1] = vtb0
                for hx in range(2):
                    nc.sync.dma_start(
                        out=kts[0][:, hx * (S // 2):(hx + 1) * (S // 2)],
                        in_=kv_pair[0][hx, 0])
                    nc.sync.dma_start(
                        out=vtb0[:, hx * (S // 2):(hx + 1) * (S // 2)],
                        in_=kv_pair[0][hx, 1])
                    nc.sync.dma_start(
                        out=kts[1][:, hx * (S // 2):(hx + 1) * (S // 2)],
                        in_=kv_pair[1][hx, 0])

              _mark(nc, "B0")
              # ------- phase B: causal attention (h outer, batch inner) -------
              # One-pair-ahead software pipeline: scores(p+1) are emitted
              # before attn@V(p) so the PE never waits on exp(p) to make
              # progress, and the scalar engine runs exp back-to-back.
              with tc.tile_pool(name="pbe", bufs=3) as pbe, \
                   tc.tile_pool(name="pbd", bufs=1) as pbd, \
                   tc.tile_pool(name="pbn", bufs=2) as pbn, \
                   tc.tile_pool(name="pbsc", bufs=2, space="PSUM") as pbsc, \
                   tc.tile_pool(name="pbo", bufs=2, space="PSUM") as pbo, \
                   tc.tile_pool(name="pbs", bufs=1, space="PSUM") as pbs, \
                   tc.tile_pool(name="patrB", bufs=1, space="PSUM") as patrB:
                mybir_exp = mybir.ActivationFunctionType.Exp

                # dedicated diagonal-pair et tiles: the fully-masked column
                # ranges are zeroed ONCE here and never written again (exp
                # only writes the live ranges).
                etdA = pbd.tile([128, 2 * SQ], dt.bfloat16, name="etdA")
                etdB = pbd.tile([128, 2 * SQ], dt.bfloat16, name="etdB")
                nc.vector.memzero(etdA[:, SQ:SQ + KV])
                nc.vector.memzero(etdB[:, 0:2 * KV])
                nc.vector.memzero(etdB[:, SQ:SQ + 3 * KV])
                # ps/pbc share one PSUM bank: row 0 holds the ones-matmul
                # column sums, then the broadcast matmul overwrites all 128
                # rows with 1/sum (after the reciprocal has read row 0).
                pspbc = pbs.tile([128, SQ], dt.float32, name="pspbc")

                sections = [(h, b) for h in range(HPC) for b in range(B)]
                tasks = []
                for sidx, (h, b) in enumerate(sections):
                    for si in range(NB):
                        for j2 in range((si + 1) * DIAG // 2):
                            tasks.append((sidx, si, j2))

                done_T = set()
                psc_of = {}
                aobs = {}

                def emit_T(b):
                    # V arrives transposed [HD, kv]; transpose to natural
                    # layout via PE, two transposes per PSUM tile so the
                    # PE->DVE chain pipelines within one bank.
                    vtb, vnb = vtbs[b], vns[b]
                    ptr = patrB.tile([128, 2 * 128], dt.bfloat16, name="ptr",
                                     tag="ptr")
                    for j in range(NKV_B):
                        half = (j % 2) * 128
                        nc.tensor.transpose(ptr[:, half:half + 128],
                                            vtb[:, j * 128:(j + 1) * 128],
                                            id_sb[:])
                        # batch 0's copies run at the A/B seam where the DVE
                        # is still draining RoPE (use the idle scalar engine);
                        # batch 1's run mid-phase-B where ACT is saturated
                        # (use the DVE's slack).
                        if b == 0:
                            nc.scalar.copy(out=vnb[:, j, :],
                                           in_=ptr[:, half:half + 128])
                        else:
                            nc.vector.tensor_copy(out=vnb[:, j, :],
                                                  in_=ptr[:, half:half + 128])
                    done_T.add(b)
                    if b == 0:
                        # batch 1's V reuses the buffer once b0 is transposed
                        for hx in range(2):
                            nc.sync.dma_start(
                                out=vtb[:, hx * (S // 2):(hx + 1) * (S // 2)],
                                in_=kv_pair[1][hx, 1])

                def emit_scores(t):
                    sidx, si, j2 = t
                    h, b = sections[sidx]
                    if b not in done_T:
                        emit_T(b)
                    qt, kt = qts[b], kts[b]
                    psc = pbsc.tile([128, 2 * SQ], dt.float32, name="psc",
                                    tag="psc")
                    ndiag = si * DIAG          # first diagonal kv-block
                    for jj in range(2):
                        j = j2 * 2 + jj
                        dd = j - ndiag
                        half = jj * SQ
                        nc.tensor.matmul(
                            psc[:, half:half + SQ],
                            lhsT=kt[:, j * KV:(j + 1) * KV],
                            rhs=qt[:, h, si * SQ:(si + 1) * SQ],
                            start=True, stop=(dd < 0))
                        if dd >= 0:
                            # strict-upper -1e9 on the diagonal 128-block:
                            # exp underflows to exact 0 there.
                            nc.tensor.matmul(
                                psc[:, half + dd * KV:half + (dd + 1) * KV],
                                lhsT=tri_sb[:], rhs=id_sb[:],
                                start=False, stop=True, skip_group_check=True)
                    psc_of[t] = psc

                def emit_exp(t, psc):
                    sidx, si, j2 = t
                    ndiag2 = si * DIAG // 2
                    if j2 < ndiag2:
                        et = pbe.tile([128, 2 * SQ], dt.bfloat16, name="et",
                                      tag="et")
                        nc.scalar.activation(et[:], psc[:], mybir_exp,
                                             scale=SCALE)
                    elif j2 == ndiag2:          # diagonal pair A (dd=0,1)
                        et = etdA
                        nc.scalar.activation(et[:, 0:SQ], psc[:, 0:SQ],
                                             mybir_exp, scale=SCALE)
                        nc.scalar.activation(et[:, SQ + KV:2 * SQ],
                                             psc[:, SQ + KV:2 * SQ],
                                             mybir_exp, scale=SCALE)
                    else:                       # diagonal pair B (dd=2,3)
                        et = etdB
                        nc.scalar.activation(et[:, 2 * KV:SQ],
                                             psc[:, 2 * KV:SQ],
                                             mybir_exp, scale=SCALE)
                        nc.scalar.activation(et[:, SQ + 3 * KV:2 * SQ],
                                             psc[:, SQ + 3 * KV:2 * SQ],
                                             mybir_exp, scale=SCALE)
                    return et

                def emit_acc(t, et, acc):
                    _, si, j2 = t
                    if j2 == 0:
                        nc.vector.tensor_add(out=acc[:], in0=et[:, 0:SQ],
                                             in1=et[:, SQ:2 * SQ])
                    else:
                        nc.vector.tensor_add(out=acc[:], in0=acc[:],
                                             in1=et[:, 0:SQ])
                        nc.vector.tensor_add(out=acc[:], in0=acc[:],
                                             in1=et[:, SQ:2 * SQ])

                def emit_av(t, et, po):
                    sidx, si, j2 = t
                    h, b = sections[sidx]
                    vn = vns[b]
                    nkv = (si + 1) * DIAG
                    for jj in range(2):
                        j = j2 * 2 + jj
                        nc.tensor.matmul(po[:], lhsT=vn[:, j, :],
                                         rhs=et[:, jj * SQ:(jj + 1) * SQ],
                                         start=(j == 0), stop=(j == nkv - 1))

                # deferred normalize: stage1 (ones-matmul + reciprocal) in the
                # next q-block's first pair window, stage2 (broadcast matmul +
                # PSUM-direct multiply + per-q-block aob ship) in the second.
                def norm1(po, acc, hh, bb, si):
                    nc.tensor.matmul(pspbc[0:1, :], lhsT=ob_sb[:], rhs=acc[:],
                                     start=True, stop=True)
                    rec = pbn.tile([1, SQ], dt.bfloat16, name="rec", tag="rec")
                    with nc.allow_low_precision(
                            reason="bf16 1/sum: 0.1% common-mode noise"):
                        nc.vector.reciprocal(out=rec[:], in_=pspbc[0:1, :])
                    return rec

                def norm2(po, acc, hh, bb, si, rec):
                    nc.tensor.matmul(pspbc[:], lhsT=of_sb[:], rhs=rec[:],
                                     start=True, stop=True)
                    # the DVE cannot read two PSUM operands in one op: stage
                    # the broadcast through SBUF
                    bc = pbn.tile([128, SQ], dt.float32, name="bc", tag="bc")
                    nc.vector.tensor_copy(out=bc[:], in_=pspbc[:])
                    aob = pbn.tile([HD, SQ], dt.bfloat16, name="aob",
                                   tag="aob", bufs=3)
                    nc.vector.tensor_mul(out=aob[:], in0=po[:], in1=bc[:])
                    g0 = bb * S + si * SQ
                    nc.sync.dma_start(
                        out=ao_in[hh][g0 // SHARD, :,
                                      g0 % SHARD:g0 % SHARD + SQ],
                        in_=aob[:])
                    if bb == B - 1 and si == NB - 1 and not sim_mode:
                        nc.gpsimd.collective_compute(
                            "AllToAll", mybir.AluOpType.bypass,
                            replica_groups=[list(range(N_CORES))],
                            ins=[ao_in[hh][:]],
                            outs=[ao_ex[hh][:]])

                pending = None
                po = acc = None
                emit_scores(tasks[0])
                for ti, t in enumerate(tasks):
                    sidx, si, j2 = t
                    h, b = sections[sidx]
                    if j2 == 0:
                        if si == 0 and sidx == HPC * B - 1:
                            # h0's AllToAll has completed by now: stream the
                            # even-head context into SBUF during this section
                            nc.sync.dma_start(
                                out=lt_all[0][:],
                                in_=ao_ex[0][:].rearrange(
                                    "a p (s n) -> p s a n", n=128))
                        po = pbo.tile([HD, SQ], dt.float32, name="po", tag="po")
                        acc = pbe.tile([128, SQ], dt.bfloat16, name="acc",
                                       tag="acc")
                    psc = psc_of.pop(t)
                    # PE stream: next pair's scores first, then any pending
                    # normalize matmul, then this pair's attn@V.
                    if ti + 1 < len(tasks):
                        emit_scores(tasks[ti + 1])
                    if j2 == 0 and pending is not None:
                        rec = norm1(*pending)
                        pending = (*pending, rec)
                    elif j2 == 1 and pending is not None and len(pending) == 6:
                        norm2(*pending)
                        pending = None
                    et = emit_exp(t, psc)
                    emit_acc(t, et, acc)
                    emit_av(t, et, po)
                    if j2 == (si + 1) * DIAG // 2 - 1:   # last pair of q-block
                        pending = (po, acc, h, b, si)
                if pending is not None:
                    rec = norm1(*pending)
                    norm2(*pending, rec)
                    pending = None

            _qkv_cm.__exit__(None, None, None)

            _mark(nc, "D")
            # ------- phase D: o_proj on this core's row shard -------
            # context is already resident (pdp pool, streamed during phase
            # B); o_proj weights stream per output-column quarter (dj-major
            # loop) with the next quarter prefetching behind the current
            # one's matmuls.
            with tc.tile_pool(name="pd", bufs=4) as pd, \
                 tc.tile_pool(name="pdw", bufs=2) as pdw, \
                 tc.tile_pool(name="pdl", bufs=1) as pdl, \
                 tc.tile_pool(name="pdps", bufs=8, space="PSUM") as pdps:
                lt1 = pdl.tile([128, SHARD // 128, N_CORES, 128],
                               dt.bfloat16, name="lt_all1")
                lts_all = [lt_all[0], lt1]

                def load_woq(dj, split=False):
                    t = pdw.tile([128, DCH, SQ], dt.bfloat16, name="woq",
                                 tag="woq")
                    r = wo[:, dj * SQ:(dj + 1) * SQ].rearrange(
                        "(k p) m -> p k m", p=128)
                    if split:
                        # even k chunks first (the even-head pass uses k=2j);
                        # HWDGE costs ~0.6us PER DMA instruction, so keep
                        # these to two strided transfers, not per-chunk
                        nc.sync.dma_start(out=t[:, 0:DCH:2, :],
                                          in_=r[:, 0:DCH:2, :])
                        nc.sync.dma_start(out=t[:, 1:DCH:2, :],
                                          in_=r[:, 1:DCH:2, :])
                    else:
                        nc.sync.dma_start(out=t[:], in_=r)
                    return t

                woq = load_woq(0, split=True)
                # odd-head context: single 2MB transfer, fully covered by the
                # even-head pass
                nc.sync.dma_start(
                    out=lt1[:],
                    in_=ao_ex[1][:].rearrange("a p (s n) -> p s a n", n=128))
                for dj in range(D // SQ):
                    nxt = load_woq(dj + 1) if dj + 1 < D // SQ else None
                    # two passes with all 8 PSUM banks held open: the
                    # even-head pass depends only on lt_all[0] (resident) and
                    # the even wo chunks (first DMAs), covering the odd-head
                    # context/weight transfers (and, on hardware, the final
                    # AllToAll's tail)
                    pods = []
                    for si in range(SHARD // 128):
                        pod = pdps.tile([128, SQ], dt.float32, name="pod",
                                        tag="pod")
                        pods.append(pod)
                        for j in range(N_CORES):
                            nc.tensor.matmul(
                                pod[:], lhsT=lts_all[0][:, si, j, :],
                                rhs=woq[:, 2 * j, :],
                                start=(j == 0), stop=False)
                    for si in range(SHARD // 128):
                        for j in range(N_CORES):
                            nc.tensor.matmul(
                                pods[si][:], lhsT=lts_all[1][:, si, j, :],
                                rhs=woq[:, 2 * j + 1, :],
                                start=False, stop=(j == N_CORES - 1))
                        ot = pd.tile([128, SQ], dt.float32, name="ot",
                                     tag="ot")
                        nc.scalar.copy(out=ot[:], in_=pods[si][:])
                        nc.sync.dma_start(
                            out=out[si * 128:(si + 1) * 128,
                                    dj * SQ:(dj + 1) * SQ],
                            in_=ot[:])
                    woq = nxt
            _pdp_cm.__exit__(None, None, None)

    nc.compile()
    return nc


def _host_prep(x, cos, sin, wq, wk, wv, wo):
    x = np.asarray(x, dtype=np.float32)
    cos = np.asarray(cos, dtype=np.float32)
    sin = np.asarray(sin, dtype=np.float32)
    wq = np.asarray(wq, dtype=np.float32)
    wk = np.asarray(wk, dtype=np.float32)
    wv = np.asarray(wv, dtype=np.float32)
    wo = np.asarray(wo, dtype=np.float32)

    xT = np.ascontiguousarray(x.reshape(BS, D).T.astype(BF16))         # [D, BS]
    cosT = np.ascontiguousarray(cos[0].T)                              # [HD, S]
    sinT = np.ascontiguousarray(sin[0].T).copy()
    sinT[:64] = -sinT[:64]                      # fold rotate_half sign into sin

    # strict-lower -1e9 triangle: lhsT of the diagonal-block mask matmul
    rr = np.arange(128)
    tri = np.where(rr[:, None] < rr[None, :], -1e9, 0.0)
    tri = np.ascontiguousarray(tri.astype(BF16))

    ident = np.eye(128, dtype=np.float32).astype(BF16)
    onesb = np.ones((128, 1), dtype=np.float32).astype(BF16)
    onesf = np.ones((1, 128), dtype=np.float32).astype(BF16)

    wq_bf = wq.astype(BF16)
    wk_bf = wk.astype(BF16)
    wv_bf = wv.astype(BF16)
    wo_bf = np.ascontiguousarray(wo.astype(BF16))

    cos_bf = cosT.astype(BF16)
    sin_bf = sinT.astype(BF16)
    id_cos = np.ones_like(cos_bf)
    id_sin = np.zeros_like(sin_bf)

    in_maps = []
    for c in range(N_CORES):
        kvh = c // 2
        is_k_core = (c % 2 == 0)
        wkv_full = wk_bf if is_k_core else wv_bf
        in_maps.append({
            "xT": xT,
            "cosT": cos_bf,
            "sinTs": sin_bf,
            "coskv": cos_bf if is_k_core else id_cos,
            "sinkv": sin_bf if is_k_core else id_sin,
            "wq": np.ascontiguousarray(wq_bf[:, c * HPC * HD:(c + 1) * HPC * HD]),
            "wkv": np.ascontiguousarray(wkv_full[:, kvh * HD:(kvh + 1) * HD]),
            "wo": wo_bf,
            "tri": tri,
            "ident": ident,
            "onesb": onesb,
            "onesf": onesf,
        })
    return in_maps


def kernel(x, cos, sin, wq, wk, wv, wo):
    from concourse.bass_utils import run_bass_kernel_spmd

    if "nc" not in _CACHE:
        _CACHE["nc"] = _build()
    nc = _CACHE["nc"]

    in_maps = _host_prep(x, cos, sin, wq, wk, wv, wo)
    res = run_bass_kernel_spmd(nc, in_maps, core_ids=list(range(N_CORES)))
    shards = [res.results[c]["out"] for c in range(N_CORES)]
    return np.concatenate(shards, axis=0).reshape(B, S, D)
